# revision 8
# baseline (speedup 1.0000x reference)
"""GQA causal attention with ALiBi (vLLM HPUAttentionImpl prompt path) on 8 NeuronCores.

Sharding: B(2) x NKV(4) = 8 independent (batch, kv-head-group) pairs, one per core.
Each core runs flash-style attention for its 4 query heads (QPK=4) over S=2048, D=64.
ALiBi + scale are folded into the QK^T matmul via 2 augmented contraction rows:
  scores = (0.125*q)Âˇk + (-slope*i)*1 + slope*j
Causal mask is applied only on the diagonal 128x128 block (host-provided -1e9 mask);
off-diagonal upper blocks are simply never computed.
The paged KV-cache scatter outputs are pure data movement -> computed on host.
"""

import numpy as np
import sys
from contextlib import ExitStack

sys.path.insert(0, "/opt/trn_rl_repo")

import concourse.bass as bass
import concourse.bacc as bacc_mod
import concourse.mybir as mybir
from concourse.tile import TileContext
from concourse.bass_utils import run_bass_kernel_spmd

B, S = 2, 2048
NH, D = 16, 64
NKV = 4
QPK = NH // NKV          # 4 query heads per kv head (= per core)
BLOCK = 128
NBLOCKS = 64
SCALE = 0.125
P = 128                  # partition / q-tile size
T = S // P               # 16 i-tiles
F32 = mybir.dt.float32

_prog_cache = {}
TRACE = False
RUN_KWARGS = {}
LAST = {}


def _build_program():
    if "nc" in _prog_cache:
        return _prog_cache["nc"]
    nc = bacc_mod.Bacc(None, target_bir_lowering=False)
    q_d = nc.declare_dram_parameter("q", [S, QPK * D], F32, isOutput=False)
    k_d = nc.declare_dram_parameter("k", [S, D], F32, isOutput=False)
    v_d = nc.declare_dram_parameter("v", [S, D], F32, isOutput=False)
    qaug_d = nc.declare_dram_parameter("qaug", [2, QPK * S], F32, isOutput=False)
    kaug_d = nc.declare_dram_parameter("kaug", [2, S], F32, isOutput=False)
    mask_d = nc.declare_dram_parameter("mask", [P, P], F32, isOutput=False)
    eye_d = nc.declare_dram_parameter("eye", [P, P], F32, isOutputFalse := False)
    out_d = nc.declare_dram_parameter("out", [S, QPK * D], F32, isOutput=True)

    KA = D + 2  # augmented contraction dim: 64 qk rows + alibi row-term + col-term

    with TileContext(nc) as tc, ExitStack() as ctx:
        const = ctx.enter_context(tc.tile_pool(name="const", bufs=1))
        ldpool = ctx.enter_context(tc.tile_pool(name="ld", bufs=3))
        ps_t = ctx.enter_context(tc.tile_pool(name="ps_t", bufs=2, space="PSUM"))
        ps_s = ctx.enter_context(tc.tile_pool(name="ps_s", bufs=1, space="PSUM"))
        ps_o = ctx.enter_context(tc.tile_pool(name="ps_o", bufs=2, space="PSUM"))
        psb = ctx.enter_context(tc.tile_pool(name="psb", bufs=2))
        ptp = ctx.enter_context(tc.tile_pool(name="ptp", bufs=3))
        stat = ctx.enter_context(tc.tile_pool(name="stat", bufs=8))
        outp = ctx.enter_context(tc.tile_pool(name="outp", bufs=3))

        eye = const.tile([P, P], F32)
        nc.sync.dma_start(out=eye, in_=eye_d[:, :])
        mask = const.tile([P, P], F32)
        nc.sync.dma_start(out=mask, in_=mask_d[:, :])

        # staged aug rows, then DVE-copied so kT/qT have a single writer engine
        kaug_st = const.tile([2, S], F32)
        nc.sync.dma_start(out=kaug_st, in_=kaug_d[:, :])
        qaug_st = const.tile([2, QPK * S], F32)
        nc.sync.dma_start(out=qaug_st, in_=qaug_d[:, :])

        # K^T (augmented) : [KA, S]
        kT = const.tile([KA, S], F32)
        nc.vector.tensor_copy(out=kT[D : D + 2, :], in_=kaug_st)
        for jt in range(T):
            k_nat = ldpool.tile([P, D], F32, tag="knat")
            nc.sync.dma_start(out=k_nat, in_=k_d[jt * P : (jt + 1) * P, :])
            pt = ps_t.tile([P, P], F32)
            nc.tensor.transpose(pt[:D, :], k_nat, eye)
            nc.vector.tensor_copy(out=kT[:D, jt * P : (jt + 1) * P], in_=pt[:D, :])

        # Q^T (augmented; q pre-scaled by SCALE on host) per head: [KA, S] x QPK
        qT = [
            const.tile([KA, S], F32, tag=f"qT{g}", name=f"qT{g}") for g in range(QPK)
        ]
        for g in range(QPK):
            nc.vector.tensor_copy(
                out=qT[g][D : D + 2, :], in_=qaug_st[:, g * S : (g + 1) * S]
            )
        for it in range(T):
            q_nat = ldpool.tile([P, QPK * D], F32, tag="qnat")
            nc.sync.dma_start(out=q_nat, in_=q_d[it * P : (it + 1) * P, :])
            for half in range(2):
                pt = ps_t.tile([P, P], F32)
                nc.tensor.transpose(pt, q_nat[:, half * P : (half + 1) * P], eye)
                for sub in range(2):
                    g = half * 2 + sub
                    nc.vector.tensor_copy(
                        out=qT[g][:D, it * P : (it + 1) * P],
                        in_=pt[sub * D : (sub + 1) * D, :],
                    )

        # V resident in SBUF via ONE dma: [P, T, D] (j-tile jt at v_sb[:, jt, :])
        v_sb = const.tile([P, T, D], F32)
        nc.sync.dma_start(out=v_sb, in_=v_d.rearrange("(n p) d -> p n d", p=P))

        for g in range(QPK):
            for it in range(T):
                L = (it + 1) * P
                s_ps = ps_s.tile([P, S], F32)
                for jb in range((L + 511) // 512):
                    c0 = jb * 512
                    c1 = min(L, c0 + 512)
                    nc.tensor.matmul(
                        s_ps[:, c0:c1],
                        qT[g][:, it * P : (it + 1) * P],
                        kT[:, c0:c1],
                        start=True,
                        stop=True,
                    )
                # causal mask on the diagonal block
                nc.vector.tensor_tensor(
                    s_ps[:, it * P : L], s_ps[:, it * P : L], mask, mybir.AluOpType.add
                )
                neg_m = stat.tile([P, 1], F32, tag="negm")
                nc.vector.tensor_reduce(
                    neg_m, s_ps[:, :L], mybir.AxisListType.X, mybir.AluOpType.max,
                    negate=True,
                )
                p_sb = psb.tile([P, S], F32)
                rsum = stat.tile([P, 1], F32, tag="rsum")
                nc.scalar.activation(
                    p_sb[:, :L], s_ps[:, :L], mybir.ActivationFunctionType.Exp,
                    bias=neg_m, accum_out=rsum,
                )
                rinv = stat.tile([P, 1], F32, tag="rinv")
                nc.vector.reciprocal(rinv, rsum)
                o_ps = ps_o.tile([P, D], F32)
                for jt in range(it + 1):
                    pt = ps_t.tile([P, P], F32)
                    nc.tensor.transpose(pt, p_sb[:, jt * P : (jt + 1) * P], eye)
                    pt_sb = ptp.tile([P, P], F32)
                    nc.vector.tensor_copy(out=pt_sb, in_=pt)
                    nc.tensor.matmul(
                        o_ps,
                        pt_sb,
                        v_sb[:, jt, :],
                        start=(jt == 0),
                        stop=(jt == it),
                    )
                o_sb = outp.tile([P, D], F32)
                nc.vector.tensor_scalar_mul(o_sb, o_ps, rinv)
                nc.sync.dma_start(
                    out=out_d[it * P : (it + 1) * P, g * D : (g + 1) * D], in_=o_sb
                )

    nc.compile()
    _prog_cache["nc"] = nc
    return nc


def kernel(query, key, value, kv_cache, attn_bias, alibi_slopes, block_indices):
    query = np.ascontiguousarray(np.asarray(query, dtype=np.float32))
    key = np.ascontiguousarray(np.asarray(key, dtype=np.float32))
    value = np.ascontiguousarray(np.asarray(value, dtype=np.float32))
    kv_cache = np.asarray(kv_cache, dtype=np.float32)
    alibi_slopes = np.asarray(alibi_slopes, dtype=np.float32)
    block_indices = np.asarray(block_indices)

    # paged KV-cache scatter (pure data movement) on host
    key_cache = np.array(kv_cache[0])
    value_cache = np.array(kv_cache[1])
    key_cache[block_indices] = key.reshape(-1, BLOCK, NKV, D)
    value_cache[block_indices] = value.reshape(-1, BLOCK, NKV, D)

    nc = _build_program()

    ar = np.arange(S, dtype=np.float32)
    mask = np.where(
        np.arange(P)[None, :] <= np.arange(P)[:, None], 0.0, -1e9
    ).astype(np.float32)
    eye = np.eye(P, dtype=np.float32)
    kaug = np.stack([np.ones(S, dtype=np.float32), ar]).astype(np.float32)

    in_maps = []
    for c in range(8):
        b, h = divmod(c, NKV)
        sl = alibi_slopes[h * QPK : (h + 1) * QPK]
        qaug = np.empty((2, QPK * S), dtype=np.float32)
        for g in range(QPK):
            qaug[0, g * S : (g + 1) * S] = -sl[g] * ar
            qaug[1, g * S : (g + 1) * S] = sl[g]
        in_maps.append(
            {
                "q": np.ascontiguousarray(
                    query[b, :, h * QPK * D : (h + 1) * QPK * D] * SCALE
                ),
                "k": np.ascontiguousarray(key[b, :, h * D : (h + 1) * D]),
                "v": np.ascontiguousarray(value[b, :, h * D : (h + 1) * D]),
                "qaug": qaug,
                "kaug": kaug,
                "mask": mask,
                "eye": eye,
            }
        )

    res = run_bass_kernel_spmd(
        nc, in_maps, list(range(8)), trace=TRACE, **RUN_KWARGS
    )
    LAST["res"] = res
    results = res.results

    out = np.empty((B, S, NH * D), dtype=np.float32)
    for c in range(8):
        b, h = divmod(c, NKV)
        out[b, :, h * QPK * D : (h + 1) * QPK * D] = results[c]["out"]

    return out, key_cache, value_cache
